# revision 5
# baseline (speedup 1.0000x reference)
"""Trainium2 Bass kernel for nn_DilatedConv (dense_cnn).

Math: the torch in-place dilated-conv loop is the affine recurrence
    s[t] = A @ s[t-1] + c[t-1],  A = weight[:, :, 0],  c[t] = W1 @ x[:, :, t+n_dil]
with s[0] = x[:, :, 0]; outputs overwrite x[:, :, 1:7936].

Since rho(A) ~ 0.74 (||A^48|| ~ 2e-6 << the 2e-2 gate), the prefix scan
truncates to a sliding window.  Define d[0] = s[0], d[1+t] = c[t]; then
s[p] = sum_k A^k d[p-k].  Per core (2 batches, data-parallel over 8 cores):

  C:    c[t] = W1 @ x[t+256]           (big matmuls, x resident in SBUF)
  up1:  l1[g] = sum_{j<16} A^{15-j} d[16g+j]   (16-tap stacked matmul, radix 16)
  win:  P[g] = s[16g+15] = sum_{m<4} A^{16m} l1[g-m]  (4-tap window; NO
        sequential scan levels at all -- A^64 is numerically zero)
  down: Horner within blocks: s[16g+i] = A s[16g+i-1] + d[16g+i], i=0..14,
        vectorized over g; the "+d" rides the PE as a 3rd (block-diagonal
        identity) matmul so PSUM->SBUF stays a pure cast, split DVE/ACT.
        Results overwrite the d slots => slot p = s[p], output DMA contiguous.
        Split in two g-halves so the first half's output DMA overlaps the
        second half's compute.

Everything device-side is bf16 (PSUM accumulation f32): halves DMA traffic,
LDWEIGHTS time, and SBUF footprint vs f32r; measured end-to-end rel err
~4.7e-3 vs the 2e-2 gate.  Host casts x to bf16, upcasts the returned
computed region, and passes through the untouched columns from the original
f32 x.
"""

import numpy as np

# ---------------- problem constants (hardcoded per spec) ----------------
B_FULL = 16
C = 256
N = 8192
N_DIL = 256
N_CORES = 8
B_LOC = B_FULL // N_CORES          # 2

N_STEPS = N - (N_DIL + 1)          # 7935 transitions; outputs cols 1..7935
DLEN = N_STEPS + 1                 # 7936 d-values (d[0]=s0, d[1+t]=c[t])
R = 16                             # block size (radix)
G = 496                            # blocks per batch (496*16 = 7936 = DLEN)
M_WIN = 4                          # window taps over block summaries
CW = DLEN + 2                      # ctile cols/batch: idx0 zero col, idx 1+p = d[p]
LW = 3 + G + 1                     # l1 cols/batch: 3 leading zero cols
XW = N - N_DIL                     # resident x cols 256..8191 (7936; last unused)
HALF_G = G // 2                    # 248 blocks per down-sweep half
HALF_T = HALF_G * R                # 3968 output cols per half

NT = 88                            # weight tiles


def _wi_w1(kc, mc): return 2 * kc + mc
def _wi_up(j, kc, mc): return 4 + 4 * j + 2 * kc + mc
def _wi_win(m, kc, mc): return 68 + 4 * m + 2 * kc + mc
def _wi_dn(kc, mc): return 84 + 2 * kc + mc


def _host_pack(weight_f32):
    """All lhsT 128x128 tiles as one (NT,128,128) bf16 array.

    matmul(out, lhsT, rhs) computes lhsT.T @ rhs, so for out = Mat @ v the
    (kc, mc) tile is Mat.T[128kc:128(kc+1), 128mc:128(mc+1)].
    """
    import ml_dtypes
    A = weight_f32[:, :, 0].astype(np.float64)
    W1 = weight_f32[:, :, 1].astype(np.float64)

    def tiles(mat):
        mt = mat.T
        return [mt[128 * kc:128 * (kc + 1), 128 * mc:128 * (mc + 1)]
                for kc in range(2) for mc in range(2)]

    Ap = [np.eye(C)]
    for _ in range(16):
        Ap.append(Ap[-1] @ A)
    pack = []
    pack += tiles(W1)
    for j in range(16):
        pack += tiles(Ap[15 - j])
    A16 = Ap[16]
    Wm = np.eye(C)
    for m in range(M_WIN):
        pack += tiles(Wm)
        Wm = Wm @ A16
    pack += tiles(A)
    assert len(pack) == NT
    return np.stack(pack, axis=0).astype(ml_dtypes.bfloat16)


def _build_program():
    import concourse.bacc as bacc
    import concourse.tile as tile
    from concourse import mybir

    bf16 = mybir.dt.bfloat16
    f32 = mybir.dt.float32

    nc = bacc.Bacc("TRN2", target_bir_lowering=False, debug=False,
                   num_devices=N_CORES)
    x_in = nc.dram_tensor("x", [B_LOC, C, N], bf16, kind="ExternalInput").ap()
    wp_in = nc.dram_tensor("wpack", [NT, 128, 128], bf16,
                           kind="ExternalInput").ap()
    out = nc.dram_tensor("out", [B_LOC, C, DLEN], bf16,
                         kind="ExternalOutput").ap()

    CHUNK = 512
    N_CH = 16                      # ceil(7935/512); last chunk is 255 wide

    with tile.TileContext(nc) as tc:
        import contextlib
        with contextlib.ExitStack() as ctx:
            wpool = ctx.enter_context(tc.tile_pool(name="wpool", bufs=1))
            cpool = ctx.enter_context(tc.tile_pool(name="cpool", bufs=1))
            xpool = ctx.enter_context(tc.tile_pool(name="xpool", bufs=1))
            lpool = ctx.enter_context(tc.tile_pool(name="lpool", bufs=1))
            pspool = ctx.enter_context(tc.tile_pool(name="ps", bufs=8,
                                                    space="PSUM"))

            wpk = wpool.tile([128, NT * 128], bf16, tag="wpk", name="wpk")
            wt = lambda i: wpk[:, 128 * i:128 * (i + 1)]
            ctile = [cpool.tile([128, B_LOC * CW], bf16, tag=f"c{mc}",
                                name=f"c{mc}") for mc in range(2)]
            xfull = [xpool.tile([128, B_LOC * XW], bf16, tag=f"x{kc}",
                                name=f"x{kc}") for kc in range(2)]
            l1 = [lpool.tile([128, B_LOC * LW], bf16, tag=f"l1_{mc}",
                             name=f"l1_{mc}") for mc in range(2)]

            # round-robin DVE/ACT for PSUM->SBUF casts
            _cp = [0]

            def copy_ps(dst, src):
                if _cp[0] % 2 == 0:
                    nc.vector.tensor_copy(dst, src)
                else:
                    nc.scalar.copy(dst, src)
                _cp[0] += 1

            # ---- loads ----
            nc.sync.dma_start(
                wpk[:, 0:4 * 128].rearrange("p (t f) -> p t f", t=4),
                wp_in[0:4].rearrange("t p f -> p t f"))
            nc.sync.dma_start(
                wpk[:, 4 * 128:].rearrange("p (t f) -> p t f", t=NT - 4),
                wp_in[4:].rearrange("t p f -> p t f"))
            # resident x: cols 256..8190 in 4 sub-DMAs per (kc, b)
            for kc in range(2):
                for b in range(B_LOC):
                    for o in range(0, XW - 1, 1984):
                        w = min(1984, (XW - 1) - o)
                        nc.sync.dma_start(
                            xfull[kc][:, b * XW + o:b * XW + o + w],
                            x_in[b, 128 * kc:128 * (kc + 1),
                                 N_DIL + o:N_DIL + o + w])
            # s0 -> d[0] slot (ctile idx 1)
            for mc in range(2):
                for b in range(B_LOC):
                    nc.sync.dma_start(
                        ctile[mc][:, b * CW + 1:b * CW + 2],
                        x_in[b, 128 * mc:128 * (mc + 1), 0:1])
            # zero cols: ctile idx 0 per batch; l1 idx 0..2 per batch
            for mc in range(2):
                for b in range(B_LOC):
                    nc.gpsimd.memset(ctile[mc][:, b * CW:b * CW + 1], 0)
                    nc.gpsimd.memset(l1[mc][:, b * LW:b * LW + 3], 0)

            # ---- phase C: c[t] = W1 @ x[t+256] -> ctile idx 2+t ----
            for b in range(B_LOC):
                for grp in range(2):
                    for mc in range(2):
                        pss, ws = [], []
                        for ch in range(8):
                            k = grp * 8 + ch
                            t0 = k * CHUNK
                            w = min(CHUNK, N_STEPS - t0)
                            if w <= 0:
                                break
                            pss.append(pspool.tile([128, CHUNK], f32,
                                                   tag="ps", name="ps"))
                            ws.append((t0, w))
                        for kc in range(2):
                            for ps, (t0, w) in zip(pss, ws):
                                nc.tensor.matmul(
                                    ps[:, :w], wt(_wi_w1(kc, mc)),
                                    xfull[kc][:, b * XW + t0:b * XW + t0 + w],
                                    start=(kc == 0), stop=(kc == 1))
                        for ps, (t0, w) in zip(pss, ws):
                            copy_ps(ctile[mc][:, b * CW + 2 + t0:
                                              b * CW + 2 + t0 + w],
                                    ps[:, :w])

            # ---- phase up1: l1[g] = sum_j A^{15-j} d[16g+j] ----
            psu = {(mc, b): pspool.tile([128, G], f32, tag="ps", name="ps")
                   for mc in range(2) for b in range(B_LOC)}
            for j in range(16):
                for kc in range(2):
                    for mc in range(2):
                        for b in range(B_LOC):
                            base = b * CW + 1 + j
                            nc.tensor.matmul(
                                psu[(mc, b)][:],
                                wt(_wi_up(j, kc, mc)),
                                ctile[kc][:, base:base + 16 * (G - 1) + 1:16],
                                start=(j == 0 and kc == 0),
                                stop=(j == 15 and kc == 1))
            for mc in range(2):
                for b in range(B_LOC):
                    copy_ps(l1[mc][:, b * LW + 3:b * LW + 3 + G],
                            psu[(mc, b)][:])

            # ---- phase win: P[g] = sum_m A^{16m} l1[g-m] -> slot 16g+15 ----
            psw = {(mc, b): pspool.tile([128, G], f32, tag="ps", name="ps")
                   for mc in range(2) for b in range(B_LOC)}
            for m in range(M_WIN):
                for kc in range(2):
                    for mc in range(2):
                        if m == 0 and kc != mc:
                            continue  # A^0 block-diagonal: off-diag tiles zero
                        for b in range(B_LOC):
                            base = b * LW + 3 - m
                            nc.tensor.matmul(
                                psw[(mc, b)][:],
                                wt(_wi_win(m, kc, mc)),
                                l1[kc][:, base:base + G],
                                start=(m == 0),
                                stop=(m == M_WIN - 1 and kc == 1))
            for mc in range(2):
                for b in range(B_LOC):
                    base = b * CW + 16
                    copy_ps(ctile[mc][:, base:base + 16 * (G - 1) + 1:16],
                            psw[(mc, b)][:])

            # ---- phase down: Horner s[16g+i] = A s[16g+i-1] + d[16g+i] ----
            # two g-halves; each half's output DMA overlaps the next half
            for half in range(2):
                gbase = half * HALF_G * R
                for i in range(15):
                    for mc in range(2):
                        for b in range(B_LOC):
                            ps = pspool.tile([128, HALF_G], f32,
                                             tag="ps", name="ps")
                            rb = b * CW + gbase + i
                            for kc in range(2):
                                nc.tensor.matmul(
                                    ps[:], wt(_wi_dn(kc, mc)),
                                    ctile[kc][:, rb:rb + 16 * (HALF_G - 1) + 1:16],
                                    start=(kc == 0), stop=False)
                            db = b * CW + gbase + i + 1
                            dslots = ctile[mc][:, db:db + 16 * (HALF_G - 1) + 1:16]
                            nc.tensor.matmul(ps[:], wt(_wi_win(0, mc, mc)),
                                             dslots, start=False, stop=True)
                            copy_ps(dslots, ps[:])
                # this half's slots are final: stream them out
                lo = half * HALF_T
                for b in range(B_LOC):
                    for mc in range(2):
                        nc.sync.dma_start(
                            out[b, 128 * mc:128 * (mc + 1), lo:lo + HALF_T],
                            ctile[mc][:, b * CW + 1 + lo:
                                      b * CW + 1 + lo + HALF_T])

    nc.compile()
    return nc


_CACHE = {}


def _get_program():
    if "nc" not in _CACHE:
        _CACHE["nc"] = _build_program()
    return _CACHE["nc"]


LAST_RESULTS = None  # test harness reads exec_time_ns off this


def kernel(x, weight, n_dil):
    import os
    import ml_dtypes
    from concourse.bass_utils import run_bass_kernel_spmd
    global LAST_RESULTS

    x = np.asarray(x)
    weight = np.asarray(weight)
    assert int(n_dil) == N_DIL and x.shape == (B_FULL, C, N)
    nc = _get_program()
    wpack = _host_pack(weight.astype(np.float32))

    xbf = x.astype(ml_dtypes.bfloat16).reshape(N_CORES, B_LOC, C, N)
    in_maps = [{"x": xbf[i], "wpack": wpack} for i in range(N_CORES)]
    trace = bool(os.environ.get("KERNEL_TRACE"))
    res = run_bass_kernel_spmd(nc, in_maps, list(range(N_CORES)), trace=trace)
    LAST_RESULTS = res
    dev = np.concatenate([res.results[i]["out"] for i in range(N_CORES)],
                         axis=0)                      # (16, 256, 7936) bf16
    out_full = x.astype(np.float32, copy=True)
    out_full[:, :, 1:1 + N_STEPS] = dev[:, :, 1:].astype(np.float32)
    return out_full.astype(x.dtype, copy=False)


# revision 6
# speedup vs baseline: 2.3412x; 2.3412x over previous
"""Trainium2 Bass kernel for nn_DilatedConv (dense_cnn).

Math: the torch in-place dilated-conv loop is the affine recurrence
    s[t] = A @ s[t-1] + c[t-1],  A = weight[:, :, 0],  c[t] = W1 @ x[:, :, t+n_dil]
with s[0] = x[:, :, 0]; outputs overwrite x[:, :, 1:7936].

Since rho(A) ~ 0.74 (||A^48|| ~ 2e-6 << the 2e-2 gate), the prefix scan
truncates to a sliding window.  Define d[0] = s[0], d[1+t] = c[t] (so
d[p] lives at block g = p//16, phase j = p%16); then s[p] = sum_k A^k d[p-k].
Per core (2 batches, data-parallel over 8 cores):

  C:    c[t] = W1 @ x[t+256]      (contiguous-rhs matmuls over 512-t chunks;
        the PSUM->SBUF copy scatters into the PHASE-MAJOR layout below)
  up1:  l1[g] = sum_{j<16} A^{15-j} d[16g+j]    (16-tap stacked matmul)
  win:  P[g] = s[16g+15] = sum_{m<4} A^{16m} l1[g-m]   (4-tap window; no
        sequential scan levels at all -- A^64 is numerically zero)
  down: Horner: s[16g+i] = A s[16g+i-1] + d[16g+i], i = 0..14, vectorized
        over g; "+d" is a DVE tensor_add fused with the PSUM->SBUF cast.
        Each step's state tile is DMA'd out as soon as it completes, so
        output DMA pipelines behind compute.

Everything SBUF-resident is PHASE-MAJOR: d/state vectors for a fixed phase j
are contiguous over g, so every matmul rhs is contiguous (strided rhs runs
~4 PE cycles/col instead of 1).  The device returns the output as
(B, C, 16, 496) [phase-major]; the host transposes back to t-order.

All device data is bf16 (PSUM accumulation f32): halves DMA, LDWEIGHTS and
SBUF vs f32r at the same PE rate; end-to-end rel err ~5e-3 vs the 2e-2 gate.
"""

import numpy as np

# ---------------- problem constants (hardcoded per spec) ----------------
B_FULL = 16
C = 256
N = 8192
N_DIL = 256
N_CORES = 8
B_LOC = B_FULL // N_CORES          # 2

N_STEPS = N - (N_DIL + 1)          # 7935 transitions; outputs cols 1..7935
DLEN = N_STEPS + 1                 # 7936 d-values
R = 16                             # block size (radix)
G = 496                            # blocks per batch (496*16 = 7936)
M_WIN = 4                          # window taps over block summaries
JW = R * G                         # 7936: phase-major d cols per batch
LW = 3 + G + 1                     # l1 cols/batch: 3 leading zero cols
SPW = 1 + G                        # P-state cols/batch: 1 leading zero col

NT = 88                            # weight tiles


def _wi_w1(kc, mc): return 2 * kc + mc
def _wi_up(j, kc, mc): return 4 + 4 * j + 2 * kc + mc
def _wi_win(m, kc, mc): return 68 + 4 * m + 2 * kc + mc
def _wi_dn(kc, mc): return 84 + 2 * kc + mc


def _host_pack(weight_f32):
    """All lhsT 128x128 tiles as one (NT,128,128) bf16 array.

    matmul(out, lhsT, rhs) computes lhsT.T @ rhs, so for out = Mat @ v the
    (kc, mc) tile is Mat.T[128kc:128(kc+1), 128mc:128(mc+1)].
    """
    import ml_dtypes
    A = weight_f32[:, :, 0].astype(np.float64)
    W1 = weight_f32[:, :, 1].astype(np.float64)

    def tiles(mat):
        mt = mat.T
        return [mt[128 * kc:128 * (kc + 1), 128 * mc:128 * (mc + 1)]
                for kc in range(2) for mc in range(2)]

    Ap = [np.eye(C)]
    for _ in range(16):
        Ap.append(Ap[-1] @ A)
    pack = []
    pack += tiles(W1)
    for j in range(16):
        pack += tiles(Ap[15 - j])
    A16 = Ap[16]
    Wm = np.eye(C)
    for m in range(M_WIN):
        pack += tiles(Wm)
        Wm = Wm @ A16
    pack += tiles(A)
    assert len(pack) == NT
    return np.stack(pack, axis=0).astype(ml_dtypes.bfloat16)


def _build_program():
    import concourse.bacc as bacc
    import concourse.tile as tile
    from concourse import mybir

    bf16 = mybir.dt.bfloat16
    f32 = mybir.dt.float32

    nc = bacc.Bacc("TRN2", target_bir_lowering=False, debug=False,
                   num_devices=N_CORES)
    x_in = nc.dram_tensor("x", [B_LOC, C, N], bf16, kind="ExternalInput").ap()
    wp_in = nc.dram_tensor("wpack", [NT, 128, 128], bf16,
                           kind="ExternalInput").ap()
    out = nc.dram_tensor("out", [B_LOC, C, R, G], bf16,
                         kind="ExternalOutput").ap()

    CHUNK = 512

    with tile.TileContext(nc) as tc:
        import contextlib
        with contextlib.ExitStack() as ctx:
            wpool = ctx.enter_context(tc.tile_pool(name="wpool", bufs=1))
            cpool = ctx.enter_context(tc.tile_pool(name="cpool", bufs=1))
            spool = ctx.enter_context(tc.tile_pool(name="spool", bufs=1))
            lpool = ctx.enter_context(tc.tile_pool(name="lpool", bufs=1))
            xwin = ctx.enter_context(tc.tile_pool(name="xwin", bufs=4))
            pspool = ctx.enter_context(tc.tile_pool(name="ps", bufs=8,
                                                    space="PSUM"))

            wpk = wpool.tile([128, NT * 128], bf16, tag="wpk", name="wpk")
            wt = lambda i: wpk[:, 128 * i:128 * (i + 1)]
            # phase-major d storage: col = b*JW + j*G + g
            cj = [cpool.tile([128, B_LOC * JW], bf16, tag=f"c{h}",
                             name=f"c{h}") for h in range(2)]
            # Horner states s[16g+i]: one tile per (i, half)
            st = [[spool.tile([128, B_LOC * G], bf16, tag=f"s{i}_{h}",
                              name=f"s{i}_{h}") for h in range(2)]
                  for i in range(15)]
            # P states (i=15) with a leading zero col per batch
            sp = [spool.tile([128, B_LOC * SPW], bf16, tag=f"sp{h}",
                             name=f"sp{h}") for h in range(2)]
            l1 = [lpool.tile([128, B_LOC * LW], bf16, tag=f"l1_{h}",
                             name=f"l1_{h}") for h in range(2)]

            # round-robin DVE/ACT for PSUM->SBUF casts
            _cp = [0]

            def copy_ps(dst, src):
                if _cp[0] % 2 == 0:
                    nc.vector.tensor_copy(dst, src)
                else:
                    nc.scalar.copy(dst, src)
                _cp[0] += 1

            # ---- loads ----
            nc.sync.dma_start(
                wpk[:, 0:4 * 128].rearrange("p (t f) -> p t f", t=4),
                wp_in[0:4].rearrange("t p f -> p t f"))
            nc.sync.dma_start(
                wpk[:, 4 * 128:].rearrange("p (t f) -> p t f", t=NT - 4),
                wp_in[4:].rearrange("t p f -> p t f"))
            # zero cols: l1 idx 0..2 and sp idx 0, per batch
            for h in range(2):
                for b in range(B_LOC):
                    nc.gpsimd.memset(l1[h][:, b * LW:b * LW + 3], 0)
                    nc.gpsimd.memset(sp[h][:, b * SPW:b * SPW + 1], 0)

            # phase-major views for the C-phase scatter copy
            cjv = [cj[h].rearrange("p (b j g) -> p b j g", b=B_LOC, j=R)
                   for h in range(2)]

            # ---- phase C: d[p] = W1 @ x[p+255] (p=0 garbage; s0 DMA'd over)
            for b in range(B_LOC):
                for k in range(16):
                    p0 = k * CHUNK
                    w = min(CHUNK, DLEN - p0)
                    nb = w // R                      # blocks in this chunk
                    xw = [xwin.tile([128, CHUNK], bf16, tag=f"xw{kc}",
                                    name=f"xw{kc}") for kc in range(2)]
                    for kc in range(2):
                        nc.sync.dma_start(
                            xw[kc][:, :w],
                            x_in[b, 128 * kc:128 * (kc + 1),
                                 p0 + 255:p0 + 255 + w])
                    for mc in range(2):
                        ps = pspool.tile([128, CHUNK], f32, tag="ps",
                                         name="ps")
                        for kc in range(2):
                            nc.tensor.matmul(ps[:, :w], wt(_wi_w1(kc, mc)),
                                             xw[kc][:, :w],
                                             start=(kc == 0), stop=(kc == 1))
                        # scatter: psum u = 16*gi + j  ->  cj[b, j, 32k+gi]
                        src = ps[:, :w].rearrange("p (g j) -> p j g", j=R)
                        dst = cjv[mc][:, b, :, 32 * k:32 * k + nb]
                        copy_ps(dst, src)
                    # s0 overwrites the p=0 garbage cell
                    if k == 0:
                        for mc in range(2):
                            nc.sync.dma_start(
                                cj[mc][:, b * JW:b * JW + 1],
                                x_in[b, 128 * mc:128 * (mc + 1), 0:1])

            # ---- phase up1: l1[g] = sum_j A^{15-j} d[16g+j] ----
            psu = {(mc, b): pspool.tile([128, G], f32, tag="ps", name="ps")
                   for mc in range(2) for b in range(B_LOC)}
            for j in range(16):
                for kc in range(2):
                    for mc in range(2):
                        for b in range(B_LOC):
                            nc.tensor.matmul(
                                psu[(mc, b)][:],
                                wt(_wi_up(j, kc, mc)),
                                cj[kc][:, b * JW + j * G:b * JW + (j + 1) * G],
                                start=(j == 0 and kc == 0),
                                stop=(j == 15 and kc == 1))
            for mc in range(2):
                for b in range(B_LOC):
                    copy_ps(l1[mc][:, b * LW + 3:b * LW + 3 + G],
                            psu[(mc, b)][:])

            # ---- phase win: P[g] = sum_m A^{16m} l1[g-m] = s[16g+15] ----
            psw = {(mc, b): pspool.tile([128, G], f32, tag="ps", name="ps")
                   for mc in range(2) for b in range(B_LOC)}
            for m in range(M_WIN):
                for kc in range(2):
                    for mc in range(2):
                        if m == 0 and kc != mc:
                            continue  # A^0 block-diagonal: off-diag zero
                        for b in range(B_LOC):
                            base = b * LW + 3 - m
                            nc.tensor.matmul(
                                psw[(mc, b)][:],
                                wt(_wi_win(m, kc, mc)),
                                l1[kc][:, base:base + G],
                                start=(m == 0),
                                stop=(m == M_WIN - 1 and kc == 1))
            for mc in range(2):
                for b in range(B_LOC):
                    copy_ps(sp[mc][:, b * SPW + 1:b * SPW + 1 + G],
                            psw[(mc, b)][:])
                    nc.sync.dma_start(
                        out[b, 128 * mc:128 * (mc + 1), 15, :],
                        sp[mc][:, b * SPW + 1:b * SPW + 1 + G])

            # ---- phase down: s[16g+i] = A s[16g+i-1] + d[16g+i], i=0..14 ----
            for i in range(15):
                for mc in range(2):
                    for b in range(B_LOC):
                        ps = pspool.tile([128, G], f32, tag="ps", name="ps")
                        for kc in range(2):
                            prev = (sp[kc][:, b * SPW:b * SPW + G] if i == 0
                                    else st[i - 1][kc][:, b * G:(b + 1) * G])
                            nc.tensor.matmul(ps[:], wt(_wi_dn(kc, mc)), prev,
                                             start=(kc == 0), stop=(kc == 1))
                        dst = st[i][mc][:, b * G:(b + 1) * G]
                        nc.vector.tensor_add(
                            dst, ps[:],
                            cj[mc][:, b * JW + i * G:b * JW + (i + 1) * G])
                        nc.sync.dma_start(
                            out[b, 128 * mc:128 * (mc + 1), i, :], dst)

    nc.compile()
    return nc


_CACHE = {}


def _get_program():
    if "nc" not in _CACHE:
        _CACHE["nc"] = _build_program()
    return _CACHE["nc"]


LAST_RESULTS = None  # test harness reads exec_time_ns off this


def kernel(x, weight, n_dil):
    import os
    import ml_dtypes
    from concourse.bass_utils import run_bass_kernel_spmd
    global LAST_RESULTS

    x = np.asarray(x)
    weight = np.asarray(weight)
    assert int(n_dil) == N_DIL and x.shape == (B_FULL, C, N)
    nc = _get_program()
    wpack = _host_pack(weight.astype(np.float32))

    xbf = x.astype(ml_dtypes.bfloat16).reshape(N_CORES, B_LOC, C, N)
    in_maps = [{"x": xbf[i], "wpack": wpack} for i in range(N_CORES)]
    trace = bool(os.environ.get("KERNEL_TRACE"))
    res = run_bass_kernel_spmd(nc, in_maps, list(range(N_CORES)), trace=trace)
    LAST_RESULTS = res
    dev = np.concatenate([res.results[i]["out"] for i in range(N_CORES)],
                         axis=0)                      # (16, 256, 16, 496) bf16
    # phase-major (i, g) -> t-major: t = 16g + i
    s_flat = dev.transpose(0, 1, 3, 2).reshape(B_FULL, C, DLEN)
    out_full = x.astype(np.float32, copy=True)
    out_full[:, :, 1:1 + N_STEPS] = s_flat[:, :, 1:].astype(np.float32)
    return out_full.astype(x.dtype, copy=False)


# revision 11
# speedup vs baseline: 2.6208x; 1.1194x over previous
"""Trainium2 Bass kernel for nn_DilatedConv (dense_cnn).

Math: the torch in-place dilated-conv loop is the affine recurrence
    s[t] = A @ s[t-1] + c[t-1],  A = weight[:, :, 0],  c[t] = W1 @ x[:, :, t+n_dil]
with s[0] = x[:, :, 0]; outputs overwrite x[:, :, 1:7936].

Since rho(A) ~ 0.74 (||A^48|| ~ 2e-6 << the 2e-2 gate), the prefix scan
truncates to a sliding window.  Define d[0] = s[0], d[1+t] = c[t] (so
d[p] lives at block g = p//16, phase j = p%16); then s[p] = sum_k A^k d[p-k].
Per core (2 batches, data-parallel over 8 cores):

  C:    c[t] = W1 @ x[t+256]      (contiguous-rhs matmuls over 512-t chunks;
        the PSUM->SBUF copy scatters into the PHASE-MAJOR layout below)
  up1:  l1[g] = sum_{j<16} A^{15-j} d[16g+j]    (16-tap stacked matmul)
  win:  P[g] = s[16g+15] = sum_{m<4} A^{16m} l1[g-m]   (4-tap window; no
        sequential scan levels at all -- A^64 is numerically zero)
  down: Horner: s[16g+i] = A s[16g+i-1] + d[16g+i], i = 0..14, vectorized
        over g; "+d" is a DVE tensor_add fused with the PSUM->SBUF cast.
        Each step's state tile is DMA'd out as soon as it completes, so
        output DMA pipelines behind compute.

Everything SBUF-resident is PHASE-MAJOR: d/state vectors for a fixed phase j
are contiguous over g, so every matmul rhs is contiguous (strided rhs runs
~4 PE cycles/col instead of 1).  The device returns the output as
(B, C, 16, 496) [phase-major]; the host transposes back to t-order.

All device data is bf16 (PSUM accumulation f32): halves DMA, LDWEIGHTS and
SBUF vs f32r at the same PE rate; end-to-end rel err ~5e-3 vs the 2e-2 gate.
"""

import numpy as np

# ---------------- problem constants (hardcoded per spec) ----------------
B_FULL = 16
C = 256
N = 8192
N_DIL = 256
N_CORES = 8
B_LOC = B_FULL // N_CORES          # 2

N_STEPS = N - (N_DIL + 1)          # 7935 transitions; outputs cols 1..7935
DLEN = N_STEPS + 1                 # 7936 d-values
R = 16                             # block size (radix)
G = 496                            # blocks per batch (496*16 = 7936)
M_WIN = 4                          # window taps over block summaries
JW = R * G                         # 7936: phase-major d cols per batch
LW = 3 + G + 1                     # l1 cols/batch: 3 leading zero cols
SPW = 1 + G                        # P-state cols/batch: 1 leading zero col

NT = 88                            # weight tiles


def _wi_w1(kc, mc): return 2 * kc + mc
def _wi_up(j, kc, mc): return 4 + 4 * j + 2 * kc + mc
def _wi_win(m, kc, mc): return 68 + 4 * m + 2 * kc + mc
def _wi_dn(kc, mc): return 84 + 2 * kc + mc


def _host_pack(weight_f32):
    """All lhsT 128x128 tiles as one (NT,128,128) bf16 array.

    matmul(out, lhsT, rhs) computes lhsT.T @ rhs, so for out = Mat @ v the
    (kc, mc) tile is Mat.T[128kc:128(kc+1), 128mc:128(mc+1)].
    """
    import ml_dtypes
    A = weight_f32[:, :, 0].astype(np.float64)
    W1 = weight_f32[:, :, 1].astype(np.float64)

    def tiles(mat):
        mt = mat.T
        return [mt[128 * kc:128 * (kc + 1), 128 * mc:128 * (mc + 1)]
                for kc in range(2) for mc in range(2)]

    Ap = [np.eye(C)]
    for _ in range(16):
        Ap.append(Ap[-1] @ A)
    pack = []
    pack += tiles(W1)
    for j in range(16):
        pack += tiles(Ap[15 - j])
    A16 = Ap[16]
    Wm = np.eye(C)
    for m in range(M_WIN):
        pack += tiles(Wm)
        Wm = Wm @ A16
    pack += tiles(A)
    assert len(pack) == NT
    return np.stack(pack, axis=0).astype(ml_dtypes.bfloat16)


def _build_program():
    import concourse.bacc as bacc
    import concourse.tile as tile
    from concourse import mybir

    bf16 = mybir.dt.bfloat16
    f32 = mybir.dt.float32

    nc = bacc.Bacc("TRN2", target_bir_lowering=False, debug=False,
                   num_devices=N_CORES)
    x_in = nc.dram_tensor("x", [B_LOC, C, N], bf16, kind="ExternalInput").ap()
    wp_in = nc.dram_tensor("wpack", [NT, 128, 128], bf16,
                           kind="ExternalInput").ap()
    out = nc.dram_tensor("out", [B_LOC, C, R, G], bf16,
                         kind="ExternalOutput").ap()

    CHUNK = 512

    with tile.TileContext(nc) as tc:
        import contextlib
        with contextlib.ExitStack() as ctx:
            wpool = ctx.enter_context(tc.tile_pool(name="wpool", bufs=1))
            cpool = ctx.enter_context(tc.tile_pool(name="cpool", bufs=1))
            spool = ctx.enter_context(tc.tile_pool(name="spool", bufs=1))
            lpool = ctx.enter_context(tc.tile_pool(name="lpool", bufs=1))
            xwin = ctx.enter_context(tc.tile_pool(name="xwin", bufs=4))
            pspool = ctx.enter_context(tc.tile_pool(name="ps", bufs=8,
                                                    space="PSUM"))

            wpk = wpool.tile([128, NT * 128], bf16, tag="wpk", name="wpk")
            wt = lambda i: wpk[:, 128 * i:128 * (i + 1)]
            # phase-major d storage: col = b*JW + j*G + g
            cj = [cpool.tile([128, B_LOC * JW], bf16, tag=f"c{h}",
                             name=f"c{h}") for h in range(2)]
            # Horner states s[16g+i]: one tile per (i, half)
            st = [[spool.tile([128, B_LOC * G], bf16, tag=f"s{i}_{h}",
                              name=f"s{i}_{h}") for h in range(2)]
                  for i in range(15)]
            # P states (i=15) with a leading zero col per batch
            sp = [spool.tile([128, B_LOC * SPW], bf16, tag=f"sp{h}",
                             name=f"sp{h}") for h in range(2)]
            l1 = [lpool.tile([128, B_LOC * LW], bf16, tag=f"l1_{h}",
                             name=f"l1_{h}") for h in range(2)]

            # round-robin DVE/ACT for PSUM->SBUF casts
            _cp = [0]

            def copy_ps(dst, src):
                if _cp[0] % 2 == 0:
                    nc.vector.tensor_copy(dst, src)
                else:
                    nc.scalar.copy(dst, src)
                _cp[0] += 1

            # ---- loads ----
            nc.sync.dma_start(
                wpk[:, 0:4 * 128].rearrange("p (t f) -> p t f", t=4),
                wp_in[0:4].rearrange("t p f -> p t f"))
            nc.sync.dma_start(
                wpk[:, 4 * 128:].rearrange("p (t f) -> p t f", t=NT - 4),
                wp_in[4:].rearrange("t p f -> p t f"))
            # zero cols: l1 idx 0..2 and sp idx 0, per batch
            for h in range(2):
                for b in range(B_LOC):
                    nc.gpsimd.memset(l1[h][:, b * LW:b * LW + 3], 0)
                    nc.gpsimd.memset(sp[h][:, b * SPW:b * SPW + 1], 0)

            # phase-major views for the C-phase scatter copy
            cjv = [cj[h].rearrange("p (b j g) -> p b j g", b=B_LOC, j=R)
                   for h in range(2)]

            # ---- phase C: d[p] = W1 @ x[p+255] (p=0 garbage; s0 DMA'd over)
            for b in range(B_LOC):
                for k in range(16):
                    p0 = k * CHUNK
                    w = min(CHUNK, DLEN - p0)
                    nb = w // R                      # blocks in this chunk
                    xw = [xwin.tile([128, CHUNK], bf16, tag=f"xw{kc}",
                                    name=f"xw{kc}") for kc in range(2)]
                    for kc in range(2):
                        eng = nc.sync if kc == 0 else nc.gpsimd
                        eng.dma_start(
                            xw[kc][:, :w],
                            x_in[b, 128 * kc:128 * (kc + 1),
                                 p0 + 255:p0 + 255 + w])
                    for mc in range(2):
                        ps = pspool.tile([128, CHUNK], f32, tag="ps",
                                         name="ps")
                        for kc in range(2):
                            nc.tensor.matmul(ps[:, :w], wt(_wi_w1(kc, mc)),
                                             xw[kc][:, :w],
                                             start=(kc == 0), stop=(kc == 1))
                        # scatter: psum u = 16*gi + j  ->  cj[b, j, 32k+gi]
                        src = ps[:, :w].rearrange("p (g j) -> p j g", j=R)
                        dst = cjv[mc][:, b, :, 32 * k:32 * k + nb]
                        copy_ps(dst, src)
                    # s0 overwrites the p=0 garbage cell
                    if k == 0:
                        for mc in range(2):
                            nc.sync.dma_start(
                                cj[mc][:, b * JW:b * JW + 1],
                                x_in[b, 128 * mc:128 * (mc + 1), 0:1])

            # ---- phase up1: l1[g] = sum_j A^{15-j} d[16g+j] ----
            psu = {(mc, b): pspool.tile([128, G], f32, tag="ps", name="ps")
                   for mc in range(2) for b in range(B_LOC)}
            for j in range(16):
                for kc in range(2):
                    for mc in range(2):
                        for b in range(B_LOC):
                            nc.tensor.matmul(
                                psu[(mc, b)][:],
                                wt(_wi_up(j, kc, mc)),
                                cj[kc][:, b * JW + j * G:b * JW + (j + 1) * G],
                                start=(j == 0 and kc == 0),
                                stop=(j == 15 and kc == 1))
            for mc in range(2):
                for b in range(B_LOC):
                    copy_ps(l1[mc][:, b * LW + 3:b * LW + 3 + G],
                            psu[(mc, b)][:])

            # ---- phase win: P[g] = sum_m A^{16m} l1[g-m] = s[16g+15] ----
            psw = {(mc, b): pspool.tile([128, G], f32, tag="ps", name="ps")
                   for mc in range(2) for b in range(B_LOC)}
            for m in range(M_WIN):
                for kc in range(2):
                    for mc in range(2):
                        if m == 0 and kc != mc:
                            continue  # A^0 block-diagonal: off-diag zero
                        for b in range(B_LOC):
                            base = b * LW + 3 - m
                            nc.tensor.matmul(
                                psw[(mc, b)][:],
                                wt(_wi_win(m, kc, mc)),
                                l1[kc][:, base:base + G],
                                start=(m == 0),
                                stop=(m == M_WIN - 1 and kc == 1))
            for mc in range(2):
                for b in range(B_LOC):
                    copy_ps(sp[mc][:, b * SPW + 1:b * SPW + 1 + G],
                            psw[(mc, b)][:])
                nc.sync.dma_start(
                    out[:, 128 * mc:128 * (mc + 1), 15, :]
                    .rearrange("b p g -> p b g"),
                    sp[mc].rearrange("p (b q) -> p b q",
                                     b=B_LOC)[:, :, 1:1 + G])

            # ---- phase down: s[16g+i] = A s[16g+i-1] + d[16g+i], i=0..14 ----
            # "+d": two lanes use a DVE tensor_add, the other two ride the PE
            # as an identity matmul + ACT cast, balancing DVE/ACT/PE load.
            dma_rr = [nc.sync, nc.gpsimd, nc.scalar]
            for i in range(15):
                for mc in range(2):
                    for b in range(B_LOC):
                        ps = pspool.tile([128, G], f32, tag="ps", name="ps")
                        dterm = cj[mc][:, b * JW + i * G:b * JW + (i + 1) * G]
                        use_dve = (mc + b) % 2 == 0
                        for kc in range(2):
                            prev = (sp[kc][:, b * SPW:b * SPW + G] if i == 0
                                    else st[i - 1][kc][:, b * G:(b + 1) * G])
                            nc.tensor.matmul(ps[:], wt(_wi_dn(kc, mc)), prev,
                                             start=(kc == 0),
                                             stop=(kc == 1 and use_dve))
                        dst = st[i][mc][:, b * G:(b + 1) * G]
                        if use_dve:
                            nc.vector.tensor_add(dst, ps[:], dterm)
                        else:
                            nc.tensor.matmul(ps[:], wt(_wi_win(0, mc, mc)),
                                             dterm, start=False, stop=True)
                            nc.scalar.copy(dst, ps[:])
                    # one DMA per (i, mc) covering both batches
                    dma_rr[(2 * i + mc) % 3].dma_start(
                        out[:, 128 * mc:128 * (mc + 1), i, :]
                        .rearrange("b p g -> p b g"),
                        st[i][mc].rearrange("p (b g) -> p b g", b=B_LOC))

    nc.compile()
    return nc


_CACHE = {}


def _get_program():
    if "nc" not in _CACHE:
        _CACHE["nc"] = _build_program()
    return _CACHE["nc"]


LAST_RESULTS = None  # test harness reads exec_time_ns off this


def kernel(x, weight, n_dil):
    import os
    import ml_dtypes
    from concourse.bass_utils import run_bass_kernel_spmd
    global LAST_RESULTS

    x = np.asarray(x)
    weight = np.asarray(weight)
    assert int(n_dil) == N_DIL and x.shape == (B_FULL, C, N)
    nc = _get_program()
    wpack = _host_pack(weight.astype(np.float32))

    xbf = x.astype(ml_dtypes.bfloat16).reshape(N_CORES, B_LOC, C, N)
    in_maps = [{"x": xbf[i], "wpack": wpack} for i in range(N_CORES)]
    trace = bool(os.environ.get("KERNEL_TRACE"))
    res = run_bass_kernel_spmd(nc, in_maps, list(range(N_CORES)), trace=trace)
    LAST_RESULTS = res
    dev = np.concatenate([res.results[i]["out"] for i in range(N_CORES)],
                         axis=0)                      # (16, 256, 16, 496) bf16
    # phase-major (i, g) -> t-major: t = 16g + i
    s_flat = dev.transpose(0, 1, 3, 2).reshape(B_FULL, C, DLEN)
    out_full = x.astype(np.float32, copy=True)
    out_full[:, :, 1:1 + N_STEPS] = s_flat[:, :, 1:].astype(np.float32)
    return out_full.astype(x.dtype, copy=False)
